# revision 30
# baseline (speedup 1.0000x reference)
"""Cached multi-head attention (dense transformer block) on 8 Trainium2 NeuronCores.

Sharding: 2 batches x 4 head-groups (8 heads each) = 8 cores.
Per core (batch b, heads h0..h0+7):
  - Q^T / K^T projected directly in [head_dim, seq] layout (lhsT = W columns,
    rhs = x^T), pair-aligned so each 128-partition block holds two heads at
    partition offsets 0 and 64 (head_dim=40, padded with zero weight columns).
  - Scores computed transposed: S^T[j, i] = K^T(:,j-chunk)^T . Q^T  -> softmax
    denominator and the P^T@V contraction both come out without any transposes.
  - V is augmented with a ones column so the AV matmul also emits sum(exp)
    per query for the softmax normalization.
  - exp() on ScalarE, everything elementwise on VectorE, softmax-reciprocal
    broadcast on GpSimd, all matmuls bf16 with fp32 PSUM accumulation.
Host: sums the 4 per-batch partial outputs (row-parallel Wo), adds bo, and
reassembles key/value caches.
"""

import time

import numpy as np
import ml_dtypes

import concourse.bass as bass
import concourse.mybir as mybir
import concourse.tile as tile
from concourse import bacc
from concourse.bass_utils import run_bass_kernel_spmd

BF = ml_dtypes.bfloat16

EMBED = 1280
NUM_HEADS = 32
HEAD_DIM = 40
SCALING = HEAD_DIM ** -0.5
BSZ, TGT, PAST = 2, 1024, 1024
SEQ = PAST + TGT
N_CORES = 8
HPC = 8          # heads per core
PAIRS = HPC // 2
KCH = EMBED // 128
P = 128

_CACHE: dict = {}


def _build_nc() -> bass.Bass:
    nc = bacc.Bacc(target_bir_lowering=False)
    dt = mybir.dt
    f32, bf = dt.float32, dt.bfloat16
    AF = mybir.ActivationFunctionType
    OP = mybir.AluOpType

    xT = nc.declare_dram_parameter("xT", [P, KCH, TGT], bf, isOutput=False)
    wq = nc.declare_dram_parameter("wq", [P, KCH, 512], bf, isOutput=False)
    wk = nc.declare_dram_parameter("wk", [P, KCH, 512], bf, isOutput=False)
    wv = nc.declare_dram_parameter("wv", [P, KCH, 320], bf, isOutput=False)
    wo = nc.declare_dram_parameter("wo", [P, PAIRS, EMBED], bf, isOutput=False)
    bqk = nc.declare_dram_parameter("bqk", [P, 2 * PAIRS], f32, isOutput=False)
    bvp = nc.declare_dram_parameter("bvp", [1, 320], f32, isOutput=False)
    pkT = nc.declare_dram_parameter("pkT", [HPC, HEAD_DIM, PAST], bf, isOutput=False)
    pv = nc.declare_dram_parameter("pv", [HPC, PAST, HEAD_DIM], bf, isOutput=False)

    out = nc.declare_dram_parameter("out", [TGT, EMBED], f32, isOutput=True)
    knew = nc.declare_dram_parameter("knew", [HPC, HEAD_DIM, TGT], bf, isOutput=True)
    vnew = nc.declare_dram_parameter("vnew", [P, HPC, 8, HEAD_DIM], bf, isOutput=True)

    with tile.TileContext(nc) as tc:
        with (
            tc.tile_pool(name="const", bufs=1) as const,
            tc.tile_pool(name="exps", bufs=6) as exps,
            tc.tile_pool(name="osb", bufs=2) as osb,
            tc.tile_pool(name="rcp", bufs=2) as rcp,
            tc.tile_pool(name="ps", bufs=4, space="PSUM") as ps,
        ):
            # input DMAs spread across engine DGE queues (sync/scalar/
            # vector/gpsimd) in need-order -- a single queue serializes at
            # ~0.7us per transfer and starves the first scores
            bqk_sb = const.tile([P, 2 * PAIRS], f32)
            nc.sync.dma_start(bqk_sb[:], bqk[:])
            xT_sb = const.tile([P, KCH, TGT], bf)
            wq_sb = const.tile([P, KCH, 512], bf)
            wk_sb = const.tile([P, KCH, 512], bf)
            for lo, hi in ((0, 3), (3, 6), (6, KCH)):
                nc.sync.dma_start(xT_sb[:, lo:hi], xT[:, lo:hi])
            for lo, hi in ((0, 5), (5, KCH)):
                nc.scalar.dma_start(wq_sb[:, lo:hi], wq[:, lo:hi])
                nc.gpsimd.dma_start(wk_sb[:, lo:hi], wk[:, lo:hi])

            vaug = const.tile([P, HPC, 16, HEAD_DIM + 1], bf)
            nc.vector.memset(vaug[:, :, :, HEAD_DIM : HEAD_DIM + 1], 1.0)
            kT = [const.tile([P, SEQ], bf, name=f"kTp{p}") for p in range(PAIRS)]
            qT = [const.tile([P, TGT], bf, name=f"qTp{p}") for p in range(PAIRS)]
            AT = [const.tile([P, TGT], bf, name=f"ATp{p}") for p in range(PAIRS)]

            for p in range(PAIRS):
                # zero pad rows first (engine partition bases must be
                # 32-aligned, so clear everything and DMA real data on top)
                nc.vector.memset(kT[p][:, 0:PAST], 0.0)
                nc.vector.memset(AT[p][:, :], 0.0)
                nc.gpsimd.dma_start(kT[p][0:40, 0:PAST], pkT[2 * p])
                nc.gpsimd.dma_start(kT[p][64:104, 0:PAST], pkT[2 * p + 1])
            for h in range(HPC):
                nc.scalar.dma_start(
                    vaug[:, h, 0:8, 0:HEAD_DIM],
                    pv[h].rearrange("(c p) d -> p c d", p=P),
                )
            wv_sb = const.tile([P, KCH, 320], bf)
            nc.scalar.dma_start(wv_sb[:], wv[:])
            bv1_sb = const.tile([1, 320], f32)
            nc.gpsimd.dma_start(bv1_sb[:], bvp[:])
            bv_bc = const.tile([P, 320], f32)
            nc.gpsimd.partition_broadcast(bv_bc[:], bv1_sb[:])
            wo_sb = const.tile([P, PAIRS, EMBED], bf)
            nc.sync.dma_start(wo_sb[:], wo[:])

            # ---- projection emitters (interleaved with attention in
            # quarter-steps so PE work stays smooth while ScalarE grinds exp)
            _proj_psum = {}

            def emit_qkt_quarter(p, quarter):
                wi, nh = divmod(quarter, 2)
                w_sb, bcol = ((wq_sb, p), (wk_sb, PAIRS + p))[wi]
                if nh == 0:
                    _proj_psum[(p, wi)] = ps.tile(
                        [P, 1024], mybir.dt.float32, tag="sc", bufs=3,
                        name=f"proj{p}_{wi}",
                    )
                psum = _proj_psum[(p, wi)]
                for k in range(KCH):
                    nc.tensor.matmul(
                        psum[:, nh * 512 : (nh + 1) * 512],
                        lhsT=w_sb[:, k, p * 128 : (p + 1) * 128],
                        rhs=xT_sb[:, k, nh * 512 : (nh + 1) * 512],
                        start=(k == 0),
                        stop=(k == KCH - 1),
                    )
                if nh == 1:
                    dst = qT[p][:, :] if bcol < PAIRS else kT[p][:, PAST:SEQ]
                    nc.vector.tensor_tensor(
                        dst,
                        psum[:, :],
                        bqk_sb[:, bcol : bcol + 1].to_broadcast([P, 1024]),
                        OP.add,
                    )

            def emit_qkt_proj(p):
                for q in range(4):
                    emit_qkt_quarter(p, q)

            def emit_v_chunk(mc):
                psum = ps.tile([P, 1024], mybir.dt.float32, tag="sc", bufs=3)
                for k in range(KCH):
                    nc.tensor.matmul(
                        psum[:, 0:320],
                        lhsT=xT_sb[:, k, mc * 128 : (mc + 1) * 128],
                        rhs=wv_sb[:, k, 0:320],
                        start=(k == 0),
                        stop=(k == KCH - 1),
                    )
                nc.vector.tensor_tensor(
                    vaug[:, :, 8 + mc, 0:HEAD_DIM],
                    psum[:, 0:320].rearrange("a (h d) -> a h d", h=HPC),
                    bv_bc[:, :].rearrange("a (h d) -> a h d", h=HPC),
                    OP.add,
                )

            # ---- attention, one head-pair at a time
            emit_qkt_proj(0)
            for p in range(PAIRS):
                # two 1-bank av tiles (one per 512-query half); head A ->
                # rows 64:105, head B -> rows 0:41 of each; disjoint
                # partitions -> independent psum accumulation groups
                av0 = ps.tile([P, 512], mybir.dt.float32, tag="av", bufs=2)
                av1 = ps.tile([P, 512], mybir.dt.float32, tag="av", bufs=2)
                avs = (av0, av1)
                for jc in range(16):
                    # keep the two heads' matmuls adjacent in the PE stream:
                    # scores A/B use disjoint row-groups (K=64 at bases 0/64)
                    # and AV A/B disjoint col-groups, so adjacent issue lets
                    # them run concurrently in the PE array
                    exs = {}
                    for base, h in ((0, 2 * p), (64, 2 * p + 1)):
                        sc = ps.tile([P, 1024], mybir.dt.float32, tag="sc", bufs=3)
                        for nh in range(2):
                            nc.tensor.matmul(
                                sc[:, nh * 512 : (nh + 1) * 512],
                                lhsT=kT[p][base : base + 64, jc * 128 : (jc + 1) * 128],
                                rhs=qT[p][base : base + 64, nh * 512 : (nh + 1) * 512],
                                start=True,
                                stop=True,
                            )
                        ex = exps.tile([P, 1024], mybir.dt.bfloat16, tag="ex")
                        nc.scalar.activation(ex[:, :], sc[:, :], AF.Exp)
                        exs[base] = ex
                    # head A -> av rows 64:105, head B -> av rows 0:41;
                    # last row of each block is sum(exp) via the ones col
                    for nh in range(2):
                        for base, h in ((0, 2 * p), (64, 2 * p + 1)):
                            obase = 64 - base
                            nc.tensor.matmul(
                                avs[nh][obase : obase + HEAD_DIM + 1, :],
                                lhsT=vaug[:, h, jc, 0 : HEAD_DIM + 1],
                                rhs=exs[base][:, nh * 512 : (nh + 1) * 512],
                                start=(jc == 0),
                                stop=(jc == 15),
                            )
                    # weave projections in AFTER this jc's attention ops so a
                    # stalled projection never head-of-line-blocks scores on
                    # the in-order PE stream. V chunk mc (emitted at jc=mc of
                    # pair 0) is first consumed at jc=8+mc; the next pair's
                    # Q/K projection is needed only at that pair's start.
                    if p == 0 and jc < 8:
                        emit_v_chunk(jc)
                    if p + 1 < PAIRS:
                        starts = (8, 10, 12, 14) if p == 0 else (2, 5, 8, 11)
                        if jc in starts:
                            emit_qkt_quarter(p + 1, starts.index(jc))
                # evacuate av to SBUF right away so the next pair's av
                # matmuls aren't blocked behind the normalize chain
                avsb = rcp.tile([P, 1024], mybir.dt.float32, tag="avsb")
                nc.vector.tensor_copy(out=avsb[0:105, 0:512], in_=av0[0:105, :])
                nc.vector.tensor_copy(out=avsb[0:105, 512:1024], in_=av1[0:105, :])
                # softmax normalize with the accumulated sum(exp).
                # The sum(exp) row (40 / 104) is not 32-aligned, so the
                # reciprocal runs on an aligned window, is DMA-shifted to
                # partition 0 (DMA has no alignment rule), and
                # partition-broadcast (base-0 only) over the out rows.
                for base in (0, 64):
                    obase = 64 - base
                    srow = obase + HEAD_DIM
                    lo = srow & ~31
                    rs = rcp.tile([P, 1024], mybir.dt.float32, tag="rs")
                    rb = rcp.tile([P, 1024], mybir.dt.float32, tag="rb")
                    nc.vector.reciprocal(rs[lo : srow + 1, :], avsb[lo : srow + 1, :])
                    nc.sync.dma_start(rs[0:1, :], rs[srow : srow + 1, :])
                    nc.gpsimd.partition_broadcast(rb[0 : obase + HEAD_DIM, :], rs[0:1, :])
                    nc.vector.tensor_tensor(
                        AT[p][obase : obase + HEAD_DIM, :],
                        avsb[obase : obase + HEAD_DIM, :],
                        rb[obase : obase + HEAD_DIM, :],
                        OP.mult,
                    )

            # new-K / new-V cache outputs (bf16; host upcasts), spread
            # across the three DMA queues
            for p in range(PAIRS):
                nc.gpsimd.dma_start(knew[2 * p], kT[p][0:40, PAST:SEQ])
                nc.scalar.dma_start(knew[2 * p + 1], kT[p][64:104, PAST:SEQ])
            for h in range(HPC):
                eng = (nc.scalar, nc.gpsimd)[h % 2]
                eng.dma_start(vnew[:, h], vaug[:, h, 8:16, 0:HEAD_DIM])

            # ---- output projection: out[i, :] += AT_pair^T @ Wo_pair
            for ic in range(8):
                po0 = ps.tile([P, 1024], mybir.dt.float32, tag="sc", bufs=3)
                po1 = ps.tile([P, 1024], mybir.dt.float32, tag="sc", bufs=3)
                for ch in range(PAIRS):
                    lhsT = AT[ch][:, ic * 128 : (ic + 1) * 128]
                    nc.tensor.matmul(
                        po0[:, 0:512], lhsT=lhsT, rhs=wo_sb[:, ch, 0:512],
                        start=(ch == 0), stop=(ch == PAIRS - 1),
                    )
                    nc.tensor.matmul(
                        po0[:, 512:1024], lhsT=lhsT, rhs=wo_sb[:, ch, 512:1024],
                        start=(ch == 0), stop=(ch == PAIRS - 1),
                    )
                    nc.tensor.matmul(
                        po1[:, 0:256], lhsT=lhsT, rhs=wo_sb[:, ch, 1024:1280],
                        start=(ch == 0), stop=(ch == PAIRS - 1),
                    )
                ob = osb.tile([P, EMBED], mybir.dt.float32, tag="ob")
                nc.vector.tensor_copy(out=ob[:, 0:1024], in_=po0[:, :])
                nc.vector.tensor_copy(out=ob[:, 1024:1280], in_=po1[:, 0:256])
                eng = (nc.sync, nc.scalar, nc.gpsimd)[ic % 3]
                eng.dma_start(out[ic * 128 : (ic + 1) * 128, :], ob[:, :])

    nc.compile()
    return nc


def _get_nc() -> bass.Bass:
    if "nc" not in _CACHE:
        _CACHE["nc"] = _build_nc()
    return _CACHE["nc"]


def _tile_k(w: np.ndarray, cols: int) -> np.ndarray:
    """[1280, cols] -> [128, 10, cols] partition-major bf16."""
    return np.ascontiguousarray(
        w.reshape(KCH, P, cols).transpose(1, 0, 2)
    ).astype(BF)


def kernel(inputs, past_key, past_value, Wq, bq, Wk, bk, Wv, bv, Wo, bo):
    inputs = np.asarray(inputs, np.float32)
    past_key = np.asarray(past_key, np.float32)
    past_value = np.asarray(past_value, np.float32)
    Wq = np.asarray(Wq, np.float32)
    bq = np.asarray(bq, np.float32)
    Wk = np.asarray(Wk, np.float32)
    bk = np.asarray(bk, np.float32)
    Wv = np.asarray(Wv, np.float32)
    bv = np.asarray(bv, np.float32)
    Wo = np.asarray(Wo, np.float32)
    bo = np.asarray(bo, np.float32)

    Wq_s = Wq * SCALING
    bq_s = bq * SCALING

    xT_tiled = {}
    for b in range(BSZ):
        xT = inputs[b].T  # [1280, 1024]
        xT_tiled[b] = np.ascontiguousarray(
            xT.reshape(KCH, P, TGT).transpose(1, 0, 2)
        ).astype(BF)

    in_maps = []
    for c in range(N_CORES):
        b, g = c // 4, c % 4
        h0 = HPC * g
        col0 = HEAD_DIM * h0  # 320*g

        # pair-padded Q/K weights: [1280, 512], pair p holds head 2p at
        # cols 0:40 and head 2p+1 at cols 64:104 of its 128-block
        wq_pad = np.zeros((EMBED, 512), np.float32)
        wk_pad = np.zeros((EMBED, 512), np.float32)
        wo_pad = np.zeros((512, EMBED), np.float32)
        bqk = np.zeros((P, 2 * PAIRS), np.float32)
        for p in range(PAIRS):
            cA = col0 + 80 * p
            wq_pad[:, 128 * p : 128 * p + 40] = Wq_s[:, cA : cA + 40]
            wq_pad[:, 128 * p + 64 : 128 * p + 104] = Wq_s[:, cA + 40 : cA + 80]
            wk_pad[:, 128 * p : 128 * p + 40] = Wk[:, cA : cA + 40]
            wk_pad[:, 128 * p + 64 : 128 * p + 104] = Wk[:, cA + 40 : cA + 80]
            # AT pair tiles hold head 2p+1 at rows 0:40 and head 2p at 64:104
            wo_pad[128 * p : 128 * p + 40, :] = Wo[cA + 40 : cA + 80, :]
            wo_pad[128 * p + 64 : 128 * p + 104, :] = Wo[cA : cA + 40, :]
            bqk[0:40, p] = bq_s[cA : cA + 40]
            bqk[64:104, p] = bq_s[cA + 40 : cA + 80]
            bqk[0:40, PAIRS + p] = bk[cA : cA + 40]
            bqk[64:104, PAIRS + p] = bk[cA + 40 : cA + 80]

        in_maps.append(
            {
                "xT": xT_tiled[b],
                "wq": _tile_k(wq_pad, 512),
                "wk": _tile_k(wk_pad, 512),
                "wv": _tile_k(Wv[:, col0 : col0 + 320], 320),
                "wo": np.ascontiguousarray(
                    wo_pad.reshape(PAIRS, P, EMBED).transpose(1, 0, 2)
                ).astype(BF),
                "bqk": bqk,
                "bvp": np.ascontiguousarray(bv[col0 : col0 + 320]).reshape(1, 320),
                "pkT": np.ascontiguousarray(
                    past_key[b, h0 : h0 + HPC].transpose(0, 2, 1)
                ).astype(BF),
                "pv": np.ascontiguousarray(past_value[b, h0 : h0 + HPC]).astype(BF),
            }
        )

    nc = _get_nc()
    # retry: a previously-wedged NeuronCore occasionally reports
    # NRT_EXEC_UNIT_UNRECOVERABLE on the first execute and recovers on rerun
    last_exc = None
    for attempt in range(3):
        try:
            results = run_bass_kernel_spmd(nc, in_maps, list(range(N_CORES))).results
            break
        except Exception as exc:  # pragma: no cover
            last_exc = exc
            time.sleep(2.0)
    else:
        raise last_exc

    attn = np.zeros((BSZ, TGT, EMBED), np.float32)
    key_states = np.empty((BSZ, NUM_HEADS, SEQ, HEAD_DIM), np.float32)
    value_states = np.empty((BSZ, NUM_HEADS, SEQ, HEAD_DIM), np.float32)
    key_states[:, :, :PAST] = past_key
    value_states[:, :, :PAST] = past_value
    for c in range(N_CORES):
        b, g = c // 4, c % 4
        h0 = HPC * g
        r = results[c]
        attn[b] += r["out"]
        # knew: [8, 40, 1024] (head, d, t) -> [8, 1024, 40]
        key_states[b, h0 : h0 + HPC, PAST:] = (
            r["knew"].astype(np.float32).transpose(0, 2, 1)
        )
        # vnew: [128, 8, 8, 40] (p, head, chunk, d) -> token = chunk*128 + p
        value_states[b, h0 : h0 + HPC, PAST:] = (
            r["vnew"].astype(np.float32).transpose(1, 2, 0, 3).reshape(HPC, TGT, HEAD_DIM)
        )
    attn += bo
    return attn, key_states, value_states


# revision 31
# speedup vs baseline: 1.0163x; 1.0163x over previous
"""Cached multi-head attention (dense transformer block) on 8 Trainium2 NeuronCores.

Sharding: 2 batches x 4 head-groups (8 heads each) = 8 cores.
Per core (batch b, heads h0..h0+7):
  - Q^T / K^T projected directly in [head_dim, seq] layout (lhsT = W columns,
    rhs = x^T), pair-aligned so each 128-partition block holds two heads at
    partition offsets 0 and 64 (head_dim=40, padded with zero weight columns).
  - Scores computed transposed: S^T[j, i] = K^T(:,j-chunk)^T . Q^T  -> softmax
    denominator and the P^T@V contraction both come out without any transposes.
  - V is augmented with a ones column so the AV matmul also emits sum(exp)
    per query for the softmax normalization.
  - exp() on ScalarE, everything elementwise on VectorE, softmax-reciprocal
    broadcast on GpSimd, all matmuls bf16 with fp32 PSUM accumulation.
Host: sums the 4 per-batch partial outputs (row-parallel Wo), adds bo, and
reassembles key/value caches.
"""

import time

import numpy as np
import ml_dtypes

import concourse.bass as bass
import concourse.mybir as mybir
import concourse.tile as tile
from concourse import bacc
from concourse.bass_utils import run_bass_kernel_spmd

BF = ml_dtypes.bfloat16

EMBED = 1280
NUM_HEADS = 32
HEAD_DIM = 40
SCALING = HEAD_DIM ** -0.5
BSZ, TGT, PAST = 2, 1024, 1024
SEQ = PAST + TGT
N_CORES = 8
HPC = 8          # heads per core
PAIRS = HPC // 2
KCH = EMBED // 128
P = 128

_CACHE: dict = {}


def _build_nc() -> bass.Bass:
    nc = bacc.Bacc(target_bir_lowering=False)
    dt = mybir.dt
    f32, bf = dt.float32, dt.bfloat16
    AF = mybir.ActivationFunctionType
    OP = mybir.AluOpType

    xT = nc.declare_dram_parameter("xT", [P, KCH, TGT], bf, isOutput=False)
    wq = nc.declare_dram_parameter("wq", [P, KCH, 512], bf, isOutput=False)
    wk = nc.declare_dram_parameter("wk", [P, KCH, 512], bf, isOutput=False)
    wv = nc.declare_dram_parameter("wv", [P, KCH, 320], bf, isOutput=False)
    wo = nc.declare_dram_parameter("wo", [P, PAIRS, EMBED], bf, isOutput=False)
    bqk = nc.declare_dram_parameter("bqk", [P, 2 * PAIRS], f32, isOutput=False)
    bvp = nc.declare_dram_parameter("bvp", [1, 320], f32, isOutput=False)
    pkT = nc.declare_dram_parameter("pkT", [HPC, HEAD_DIM, PAST], bf, isOutput=False)
    pv = nc.declare_dram_parameter("pv", [HPC, PAST, HEAD_DIM], bf, isOutput=False)

    out = nc.declare_dram_parameter("out", [TGT, EMBED], f32, isOutput=True)
    knew = nc.declare_dram_parameter("knew", [HPC, HEAD_DIM, TGT], bf, isOutput=True)
    vnew = nc.declare_dram_parameter("vnew", [P, HPC, 8, HEAD_DIM], bf, isOutput=True)

    with tile.TileContext(nc) as tc:
        with (
            tc.tile_pool(name="const", bufs=1) as const,
            tc.tile_pool(name="exps", bufs=6) as exps,
            tc.tile_pool(name="osb", bufs=3) as osb,
            tc.tile_pool(name="rcp", bufs=2) as rcp,
            tc.tile_pool(name="ps", bufs=4, space="PSUM") as ps,
        ):
            # input DMAs spread across engine DGE queues (sync/scalar/
            # vector/gpsimd) in need-order -- a single queue serializes at
            # ~0.7us per transfer and starves the first scores
            bqk_sb = const.tile([P, 2 * PAIRS], f32)
            nc.sync.dma_start(bqk_sb[:], bqk[:])
            xT_sb = const.tile([P, KCH, TGT], bf)
            wq_sb = const.tile([P, KCH, 512], bf)
            wk_sb = const.tile([P, KCH, 512], bf)
            for lo, hi in ((0, 3), (3, 6), (6, KCH)):
                nc.sync.dma_start(xT_sb[:, lo:hi], xT[:, lo:hi])
            for lo, hi in ((0, 5), (5, KCH)):
                nc.scalar.dma_start(wq_sb[:, lo:hi], wq[:, lo:hi])
                nc.gpsimd.dma_start(wk_sb[:, lo:hi], wk[:, lo:hi])

            vaug = const.tile([P, HPC, 16, HEAD_DIM + 1], bf)
            nc.vector.memset(vaug[:, :, :, HEAD_DIM : HEAD_DIM + 1], 1.0)
            kT = [const.tile([P, SEQ], bf, name=f"kTp{p}") for p in range(PAIRS)]
            qT = [const.tile([P, TGT], bf, name=f"qTp{p}") for p in range(PAIRS)]
            AT = [const.tile([P, TGT], bf, name=f"ATp{p}") for p in range(PAIRS)]

            for p in range(PAIRS):
                # zero pad rows first (engine partition bases must be
                # 32-aligned, so clear everything and DMA real data on top)
                nc.vector.memset(kT[p][:, 0:PAST], 0.0)
                nc.vector.memset(AT[p][:, :], 0.0)
                nc.gpsimd.dma_start(kT[p][0:40, 0:PAST], pkT[2 * p])
                nc.gpsimd.dma_start(kT[p][64:104, 0:PAST], pkT[2 * p + 1])
            for h in range(HPC):
                nc.scalar.dma_start(
                    vaug[:, h, 0:8, 0:HEAD_DIM],
                    pv[h].rearrange("(c p) d -> p c d", p=P),
                )
            wv_sb = const.tile([P, KCH, 320], bf)
            nc.scalar.dma_start(wv_sb[:], wv[:])
            bv1_sb = const.tile([1, 320], f32)
            nc.gpsimd.dma_start(bv1_sb[:], bvp[:])
            bv_bc = const.tile([P, 320], f32)
            nc.gpsimd.partition_broadcast(bv_bc[:], bv1_sb[:])
            wo_sb = const.tile([P, PAIRS, EMBED], bf)
            nc.sync.dma_start(wo_sb[:], wo[:])

            # ---- projection emitters (interleaved with attention in
            # quarter-steps so PE work stays smooth while ScalarE grinds exp)
            _proj_psum = {}

            def emit_qkt_quarter(p, quarter):
                wi, nh = divmod(quarter, 2)
                w_sb, bcol = ((wq_sb, p), (wk_sb, PAIRS + p))[wi]
                if nh == 0:
                    _proj_psum[(p, wi)] = ps.tile(
                        [P, 1024], mybir.dt.float32, tag="sc", bufs=3,
                        name=f"proj{p}_{wi}",
                    )
                psum = _proj_psum[(p, wi)]
                for k in range(KCH):
                    nc.tensor.matmul(
                        psum[:, nh * 512 : (nh + 1) * 512],
                        lhsT=w_sb[:, k, p * 128 : (p + 1) * 128],
                        rhs=xT_sb[:, k, nh * 512 : (nh + 1) * 512],
                        start=(k == 0),
                        stop=(k == KCH - 1),
                    )
                if nh == 1:
                    dst = qT[p][:, :] if bcol < PAIRS else kT[p][:, PAST:SEQ]
                    nc.vector.tensor_tensor(
                        dst,
                        psum[:, :],
                        bqk_sb[:, bcol : bcol + 1].to_broadcast([P, 1024]),
                        OP.add,
                    )

            def emit_qkt_proj(p):
                for q in range(4):
                    emit_qkt_quarter(p, q)

            def emit_v_chunk(mc):
                psum = ps.tile([P, 1024], mybir.dt.float32, tag="sc", bufs=3)
                for k in range(KCH):
                    nc.tensor.matmul(
                        psum[:, 0:320],
                        lhsT=xT_sb[:, k, mc * 128 : (mc + 1) * 128],
                        rhs=wv_sb[:, k, 0:320],
                        start=(k == 0),
                        stop=(k == KCH - 1),
                    )
                nc.vector.tensor_tensor(
                    vaug[:, :, 8 + mc, 0:HEAD_DIM],
                    psum[:, 0:320].rearrange("a (h d) -> a h d", h=HPC),
                    bv_bc[:, :].rearrange("a (h d) -> a h d", h=HPC),
                    OP.add,
                )

            # ---- attention, one head-pair at a time
            emit_qkt_proj(0)
            for p in range(PAIRS):
                # two 1-bank av tiles (one per 512-query half); head A ->
                # rows 64:105, head B -> rows 0:41 of each; disjoint
                # partitions -> independent psum accumulation groups
                av0 = ps.tile([P, 512], mybir.dt.float32, tag="av", bufs=2)
                av1 = ps.tile([P, 512], mybir.dt.float32, tag="av", bufs=2)
                avs = (av0, av1)
                for jc in range(16):
                    # keep the two heads' matmuls adjacent in the PE stream:
                    # scores A/B use disjoint row-groups (K=64 at bases 0/64)
                    # and AV A/B disjoint col-groups, so adjacent issue lets
                    # them run concurrently in the PE array
                    exs = {}
                    for base, h in ((0, 2 * p), (64, 2 * p + 1)):
                        sc = ps.tile([P, 1024], mybir.dt.float32, tag="sc", bufs=3)
                        for nh in range(2):
                            nc.tensor.matmul(
                                sc[:, nh * 512 : (nh + 1) * 512],
                                lhsT=kT[p][base : base + 64, jc * 128 : (jc + 1) * 128],
                                rhs=qT[p][base : base + 64, nh * 512 : (nh + 1) * 512],
                                start=True,
                                stop=True,
                            )
                        ex = exps.tile([P, 1024], mybir.dt.bfloat16, tag="ex")
                        nc.scalar.activation(ex[:, :], sc[:, :], AF.Exp)
                        exs[base] = ex
                    # head A -> av rows 64:105, head B -> av rows 0:41;
                    # last row of each block is sum(exp) via the ones col
                    for nh in range(2):
                        for base, h in ((0, 2 * p), (64, 2 * p + 1)):
                            obase = 64 - base
                            nc.tensor.matmul(
                                avs[nh][obase : obase + HEAD_DIM + 1, :],
                                lhsT=vaug[:, h, jc, 0 : HEAD_DIM + 1],
                                rhs=exs[base][:, nh * 512 : (nh + 1) * 512],
                                start=(jc == 0),
                                stop=(jc == 15),
                            )
                    # weave projections in AFTER this jc's attention ops so a
                    # stalled projection never head-of-line-blocks scores on
                    # the in-order PE stream. V chunk mc (emitted at jc=mc of
                    # pair 0) is first consumed at jc=8+mc; the next pair's
                    # Q/K projection is needed only at that pair's start.
                    if p == 0 and jc < 8:
                        emit_v_chunk(jc)
                    if p + 1 < PAIRS:
                        starts = (8, 10, 12, 14) if p == 0 else (2, 5, 8, 11)
                        if jc in starts:
                            emit_qkt_quarter(p + 1, starts.index(jc))
                # evacuate av to SBUF right away so the next pair's av
                # matmuls aren't blocked behind the normalize chain
                avsb = rcp.tile([P, 1024], mybir.dt.float32, tag="avsb")
                nc.vector.tensor_copy(out=avsb[0:105, 0:512], in_=av0[0:105, :])
                nc.vector.tensor_copy(out=avsb[0:105, 512:1024], in_=av1[0:105, :])
                # softmax normalize with the accumulated sum(exp).
                # The sum(exp) row (40 / 104) is not 32-aligned, so the
                # reciprocal runs on an aligned window, is DMA-shifted to
                # partition 0 (DMA has no alignment rule), and
                # partition-broadcast (base-0 only) over the out rows.
                for base in (0, 64):
                    obase = 64 - base
                    srow = obase + HEAD_DIM
                    lo = srow & ~31
                    rs = rcp.tile([P, 1024], mybir.dt.float32, tag="rs")
                    rb = rcp.tile([P, 1024], mybir.dt.float32, tag="rb")
                    nc.vector.reciprocal(rs[lo : srow + 1, :], avsb[lo : srow + 1, :])
                    nc.sync.dma_start(rs[0:1, :], rs[srow : srow + 1, :])
                    nc.gpsimd.partition_broadcast(rb[0 : obase + HEAD_DIM, :], rs[0:1, :])
                    nc.vector.tensor_tensor(
                        AT[p][obase : obase + HEAD_DIM, :],
                        avsb[obase : obase + HEAD_DIM, :],
                        rb[obase : obase + HEAD_DIM, :],
                        OP.mult,
                    )

            # new-K / new-V cache outputs (bf16; host upcasts), spread
            # across the three DMA queues
            for p in range(PAIRS):
                nc.gpsimd.dma_start(knew[2 * p], kT[p][0:40, PAST:SEQ])
                nc.scalar.dma_start(knew[2 * p + 1], kT[p][64:104, PAST:SEQ])
            for h in range(HPC):
                eng = (nc.scalar, nc.gpsimd)[h % 2]
                eng.dma_start(vnew[:, h], vaug[:, h, 8:16, 0:HEAD_DIM])

            # ---- output projection: out[i, :] += AT_pair^T @ Wo_pair
            for ic in range(8):
                po0 = ps.tile([P, 1024], mybir.dt.float32, tag="sc", bufs=3)
                po1 = ps.tile([P, 1024], mybir.dt.float32, tag="sc", bufs=3)
                for ch in range(PAIRS):
                    lhsT = AT[ch][:, ic * 128 : (ic + 1) * 128]
                    nc.tensor.matmul(
                        po0[:, 0:512], lhsT=lhsT, rhs=wo_sb[:, ch, 0:512],
                        start=(ch == 0), stop=(ch == PAIRS - 1),
                    )
                    nc.tensor.matmul(
                        po0[:, 512:1024], lhsT=lhsT, rhs=wo_sb[:, ch, 512:1024],
                        start=(ch == 0), stop=(ch == PAIRS - 1),
                    )
                    nc.tensor.matmul(
                        po1[:, 0:256], lhsT=lhsT, rhs=wo_sb[:, ch, 1024:1280],
                        start=(ch == 0), stop=(ch == PAIRS - 1),
                    )
                ob = osb.tile([P, EMBED], mybir.dt.float32, tag="ob")
                nc.vector.tensor_copy(out=ob[:, 0:1024], in_=po0[:, :])
                nc.scalar.copy(ob[:, 1024:1280], po1[:, 0:256])
                eng = (nc.sync, nc.scalar, nc.gpsimd)[ic % 3]
                eng.dma_start(out[ic * 128 : (ic + 1) * 128, :], ob[:, :])

    nc.compile()
    return nc


def _get_nc() -> bass.Bass:
    if "nc" not in _CACHE:
        _CACHE["nc"] = _build_nc()
    return _CACHE["nc"]


def _tile_k(w: np.ndarray, cols: int) -> np.ndarray:
    """[1280, cols] -> [128, 10, cols] partition-major bf16."""
    return np.ascontiguousarray(
        w.reshape(KCH, P, cols).transpose(1, 0, 2)
    ).astype(BF)


def kernel(inputs, past_key, past_value, Wq, bq, Wk, bk, Wv, bv, Wo, bo):
    inputs = np.asarray(inputs, np.float32)
    past_key = np.asarray(past_key, np.float32)
    past_value = np.asarray(past_value, np.float32)
    Wq = np.asarray(Wq, np.float32)
    bq = np.asarray(bq, np.float32)
    Wk = np.asarray(Wk, np.float32)
    bk = np.asarray(bk, np.float32)
    Wv = np.asarray(Wv, np.float32)
    bv = np.asarray(bv, np.float32)
    Wo = np.asarray(Wo, np.float32)
    bo = np.asarray(bo, np.float32)

    Wq_s = Wq * SCALING
    bq_s = bq * SCALING

    xT_tiled = {}
    for b in range(BSZ):
        xT = inputs[b].T  # [1280, 1024]
        xT_tiled[b] = np.ascontiguousarray(
            xT.reshape(KCH, P, TGT).transpose(1, 0, 2)
        ).astype(BF)

    in_maps = []
    for c in range(N_CORES):
        b, g = c // 4, c % 4
        h0 = HPC * g
        col0 = HEAD_DIM * h0  # 320*g

        # pair-padded Q/K weights: [1280, 512], pair p holds head 2p at
        # cols 0:40 and head 2p+1 at cols 64:104 of its 128-block
        wq_pad = np.zeros((EMBED, 512), np.float32)
        wk_pad = np.zeros((EMBED, 512), np.float32)
        wo_pad = np.zeros((512, EMBED), np.float32)
        bqk = np.zeros((P, 2 * PAIRS), np.float32)
        for p in range(PAIRS):
            cA = col0 + 80 * p
            wq_pad[:, 128 * p : 128 * p + 40] = Wq_s[:, cA : cA + 40]
            wq_pad[:, 128 * p + 64 : 128 * p + 104] = Wq_s[:, cA + 40 : cA + 80]
            wk_pad[:, 128 * p : 128 * p + 40] = Wk[:, cA : cA + 40]
            wk_pad[:, 128 * p + 64 : 128 * p + 104] = Wk[:, cA + 40 : cA + 80]
            # AT pair tiles hold head 2p+1 at rows 0:40 and head 2p at 64:104
            wo_pad[128 * p : 128 * p + 40, :] = Wo[cA + 40 : cA + 80, :]
            wo_pad[128 * p + 64 : 128 * p + 104, :] = Wo[cA : cA + 40, :]
            bqk[0:40, p] = bq_s[cA : cA + 40]
            bqk[64:104, p] = bq_s[cA + 40 : cA + 80]
            bqk[0:40, PAIRS + p] = bk[cA : cA + 40]
            bqk[64:104, PAIRS + p] = bk[cA + 40 : cA + 80]

        in_maps.append(
            {
                "xT": xT_tiled[b],
                "wq": _tile_k(wq_pad, 512),
                "wk": _tile_k(wk_pad, 512),
                "wv": _tile_k(Wv[:, col0 : col0 + 320], 320),
                "wo": np.ascontiguousarray(
                    wo_pad.reshape(PAIRS, P, EMBED).transpose(1, 0, 2)
                ).astype(BF),
                "bqk": bqk,
                "bvp": np.ascontiguousarray(bv[col0 : col0 + 320]).reshape(1, 320),
                "pkT": np.ascontiguousarray(
                    past_key[b, h0 : h0 + HPC].transpose(0, 2, 1)
                ).astype(BF),
                "pv": np.ascontiguousarray(past_value[b, h0 : h0 + HPC]).astype(BF),
            }
        )

    nc = _get_nc()
    # retry: a previously-wedged NeuronCore occasionally reports
    # NRT_EXEC_UNIT_UNRECOVERABLE on the first execute and recovers on rerun
    last_exc = None
    for attempt in range(3):
        try:
            results = run_bass_kernel_spmd(nc, in_maps, list(range(N_CORES))).results
            break
        except Exception as exc:  # pragma: no cover
            last_exc = exc
            time.sleep(2.0)
    else:
        raise last_exc

    attn = np.zeros((BSZ, TGT, EMBED), np.float32)
    key_states = np.empty((BSZ, NUM_HEADS, SEQ, HEAD_DIM), np.float32)
    value_states = np.empty((BSZ, NUM_HEADS, SEQ, HEAD_DIM), np.float32)
    key_states[:, :, :PAST] = past_key
    value_states[:, :, :PAST] = past_value
    for c in range(N_CORES):
        b, g = c // 4, c % 4
        h0 = HPC * g
        r = results[c]
        attn[b] += r["out"]
        # knew: [8, 40, 1024] (head, d, t) -> [8, 1024, 40]
        key_states[b, h0 : h0 + HPC, PAST:] = (
            r["knew"].astype(np.float32).transpose(0, 2, 1)
        )
        # vnew: [128, 8, 8, 40] (p, head, chunk, d) -> token = chunk*128 + p
        value_states[b, h0 : h0 + HPC, PAST:] = (
            r["vnew"].astype(np.float32).transpose(1, 2, 0, 3).reshape(HPC, TGT, HEAD_DIM)
        )
    attn += bo
    return attn, key_states, value_states
